# revision 1
# baseline (speedup 1.0000x reference)
"""Self-contained Trainium2 Bass kernel for a batched (time-stepped) GAT layer.

Problem: x [N=20000, T=8, F=128], edge_index [2, E=320000] (+self loops),
W [128, 256] (4 heads x 64), att_src/att_dst [4, 64], bias [64].
Per time step: GATConv (concat=False -> head mean) with softmax attention.
Output: [N, T, 64] f32.

Sharding: data-parallel over the T=8 time steps across 8 NeuronCores
(each step independent given shared weights; weights replicated).

Per-core algorithm (one time step):
  Phase 1 (dense): h = x_t @ W_aug (W augmented with 8 columns so the same
    matmul yields per-node a_src/a_dst attention logits). h is written to an
    HBM side array 'hext' ([N, 384] bf16 rows: 256 bf16 h | 4 f32 a_src | pad)
    rows also carry a_dst; a 256B tail slice of the same rows serves the
    dst-indexed gather).
  Phase 2 (edges, sorted by destination on host, chunked 128/dst-tile):
    - dma_gather hext rows by src  -> h[src], a_src[src]
    - dma_gather hext tail slices by dst -> a_dst[dst] (pad edges hit a
      dummy row with a_dst=-1000 so exp(alpha) == 0)
    - alpha = leaky_relu(a_src+a_dst); ex = exp(alpha)   (softmax max-shift is
      unnecessary: |alpha| <= ~10, exp stays in f32 range; softmax invariant)
    - msg = h[src] * ex (broadcast per head) ++ ex columns
    - one-hot(dst_local) matmul accumulates segment sums into PSUM:
      numerator [128, 256] and denominators [128, 4] in one [128, 260] matmul
    - out = (numerator / denom).mean(heads) + bias
"""

import numpy as np
import ml_dtypes
from contextlib import ExitStack

import concourse.bass as bass
import concourse.bacc as bacc
import concourse.mybir as mybir
import concourse.tile as tile
from concourse import library_config
from concourse.bass_utils import run_bass_kernel_spmd

F32 = mybir.dt.float32
BF16 = mybir.dt.bfloat16
FP16 = mybir.dt.float16
I16 = mybir.dt.int16

P = 128


class GatConfig:
    def __init__(self, n_nodes, in_dim, heads, d_model, neg_slope):
        self.n_nodes = n_nodes
        self.in_dim = in_dim
        self.heads = heads
        self.d_model = d_model
        self.hc = heads * d_model
        self.neg_slope = neg_slope
        self.n_tiles = (n_nodes + P - 1) // P
        self.n_pad = self.n_tiles * P
        self.dummy_row = self.n_pad            # dummy hext row for padded edges
        self.hext_rows = self.n_pad + P
        # bf16 cols: hc h | 8 (a_src f32) | 8 (a_dst f32) | pad to hc+128
        self.hext_w = self.hc + P
        self.aux_w = P                         # gather2: bf16 cols hc..hc+128
        self.mm_w = self.hc + heads            # matmul rhs width (msg | ex)


CFG = GatConfig(n_nodes=20000, in_dim=128, heads=4, d_model=64, neg_slope=0.2)
T_STEPS = 8
N_CORES = 8


def preprocess_edges(cfg, edge_index):
    """Sort (edges + self loops) by destination, pad each 128-node dst tile's
    edge list to a multiple of 128, and produce wrapped int16 gather indices.

    Returns (g1_wrapped, g2_wrapped, chunks_per_tile).
    g1: source-node index per edge slot (pad slots -> 0, harmless: ex==0).
    g2: dst-node index per edge slot (pad slots -> dummy aux row).
    Wrapped layout: flat slot j lives at [j % 16, j // 16], replicated to
    128 partitions (8 copies of the 16-partition group) as HW requires.
    """
    n = cfg.n_nodes
    loops = np.arange(n, dtype=np.int64)
    src = np.concatenate([np.asarray(edge_index[0], dtype=np.int64), loops])
    dst = np.concatenate([np.asarray(edge_index[1], dtype=np.int64), loops])
    order = np.argsort(dst, kind="stable")
    src_s = src[order]
    dst_s = dst[order]
    counts = np.bincount(dst_s // P, minlength=cfg.n_tiles)
    g1_parts, g2_parts, chunks = [], [], []
    pos = 0
    for m in range(cfg.n_tiles):
        length = int(counts[m])
        lpad = max(P, ((length + P - 1) // P) * P)
        g1 = np.zeros(lpad, np.int16)
        g2 = np.full(lpad, cfg.dummy_row, np.int16)
        g1[:length] = src_s[pos : pos + length]
        g2[:length] = dst_s[pos : pos + length]
        g1_parts.append(g1)
        g2_parts.append(g2)
        chunks.append(lpad // P)
        pos += length
    assert pos == src_s.size

    def wrap(flat):
        w = flat.reshape(-1, 16).T.copy()       # [16, E_pad/16]
        return np.tile(w, (8, 1)).copy()        # [128, E_pad/16]

    g1_all = np.concatenate(g1_parts)
    g2_all = np.concatenate(g2_parts)
    # dst_local per edge slot, laid out [128 lanes, chunk]: pad slots get 200
    # (matches no one-hot row -> padded edges contribute nothing).
    dl_flat = np.where(
        g2_all == cfg.dummy_row, 200.0, (g2_all.astype(np.int64) % P).astype(np.float64)
    )
    dl_all = dl_flat.reshape(-1, P).T.astype(ml_dtypes.bfloat16).copy()
    return wrap(g1_all), wrap(g2_all), chunks, dl_all


def build_consts(cfg, W, att_src, att_dst, bias):
    """Host-side constant tensors shared by all cores."""
    W = np.asarray(W, np.float32)
    att_src = np.asarray(att_src, np.float32)
    att_dst = np.asarray(att_dst, np.float32)
    bias = np.asarray(bias, np.float32)
    Wr = W.reshape(cfg.in_dim, cfg.heads, cfg.d_model)
    a_src_cols = np.einsum("fhc,hc->fh", Wr, att_src)
    a_dst_cols = np.einsum("fhc,hc->fh", Wr, att_dst)
    # h channels stored (c, h)-major: col = c*H + h. Keeps the per-head ex
    # broadcast AP's innermost step at 1 (DVE 2x-mode packable).
    W_perm = np.ascontiguousarray(
        Wr.transpose(0, 2, 1).reshape(cfg.in_dim, cfg.hc)
    )
    waug = np.concatenate([W_perm, a_src_cols, a_dst_cols], axis=1)
    biasrep = np.tile(bias[None, :], (P, 1)).astype(np.float32)
    t2row = np.tile(
        np.arange(P, dtype=ml_dtypes.bfloat16)[None, :], (P, 1)
    ).copy()
    ident = np.eye(P, dtype=np.float32)
    return {
        "waug": np.ascontiguousarray(waug, np.float32),
        "biasrep": biasrep,
        "t2row": t2row,
        "ident": ident,
    }


def build_nc(cfg, chunks, e16, debug=False, num_devices=N_CORES):
    """Build the full Bass program (SPMD: identical across cores)."""
    nc = bacc.Bacc(
        "TRN2",
        target_bir_lowering=False,
        debug=debug,
        num_devices=num_devices,
        num_swdge_queues=4,
    )
    n_chunks_tot = sum(chunks)
    heads, hc = cfg.heads, cfg.hc
    naug = hc + 2 * heads

    xt = nc.dram_tensor("xt", [cfg.n_nodes, cfg.in_dim], F32, kind="ExternalInput")
    waug = nc.dram_tensor("waug", [cfg.in_dim, naug], F32, kind="ExternalInput")
    biasrep = nc.dram_tensor("biasrep", [P, cfg.d_model], F32, kind="ExternalInput")
    t2row = nc.dram_tensor("t2row", [P, P], BF16, kind="ExternalInput")
    dl = nc.dram_tensor("dl", [P, n_chunks_tot], BF16, kind="ExternalInput")
    ident = nc.dram_tensor("ident", [P, P], F32, kind="ExternalInput")
    g1 = nc.dram_tensor("g1", [P, e16], I16, kind="ExternalInput")
    g2 = nc.dram_tensor("g2", [P, e16], I16, kind="ExternalInput")
    hext = nc.dram_tensor("hext", [cfg.hext_rows, cfg.hext_w], BF16, kind="Internal")
    out = nc.dram_tensor("out", [cfg.n_nodes, cfg.d_model], F32, kind="ExternalOutput")

    with tile.TileContext(nc) as tc, ExitStack() as ctx:
        nc.gpsimd.load_library(library_config.mlp)
        tc.no_sync_barrier()

        consts = ctx.enter_context(tc.tile_pool(name="consts", bufs=1))
        waug_f32 = consts.tile([P, naug], F32)
        nc.sync.dma_start(waug_f32[:], waug[:, :])
        waug_t = consts.tile([P, naug], BF16)
        nc.vector.tensor_copy(waug_t[:], waug_f32[:])
        bias_t = consts.tile([P, cfg.d_model], F32)
        nc.sync.dma_start(bias_t[:], biasrep[:, :])
        t2_t = consts.tile([P, P], BF16)
        nc.sync.dma_start(t2_t[:], t2row[:, :])
        id_t = consts.tile([P, P], F32)
        nc.sync.dma_start(id_t[:], ident[:, :])

        # dst_local constants are tiny — keep resident; gather indices are
        # streamed per tile (88KB resident would crowd out double-buffering).
        idxpool = ctx.enter_context(tc.tile_pool(name="idx", bufs=1))
        dls = idxpool.tile([P, n_chunks_tot], BF16)
        nc.sync.dma_start(dls[:], dl[:, :])

        # ---------------- phase 1: dense h + logits ----------------
        h_scope = nc.enter_named_scope("h_phase", False)[0]
        xpool = ctx.enter_context(tc.tile_pool(name="x", bufs=3))
        stpool = ctx.enter_context(tc.tile_pool(name="stage", bufs=3))
        ps_tr = ctx.enter_context(tc.tile_pool(name="ps_tr", bufs=2, space="PSUM"))
        ps_h = ctx.enter_context(tc.tile_pool(name="ps_h", bufs=2, space="PSUM"))

        for m in range(cfg.n_tiles):
            n0 = m * P
            nrows = min(P, cfg.n_nodes - n0)
            xtile = xpool.tile([P, cfg.in_dim], F32, tag="xtile")
            if nrows < P:
                nc.vector.memset(xtile[:], 0.0)
            nc.sync.dma_start(xtile[:nrows, :], xt[n0 : n0 + nrows, :])
            ptr = ps_tr.tile([P, P], F32)
            nc.tensor.transpose(ptr[:], xtile[:], id_t[:])
            xT = xpool.tile([P, P], BF16, tag="xT")
            nc.vector.tensor_copy(xT[:], ptr[:])
            ph = ps_h.tile([P, naug], F32)
            nc.tensor.matmul(ph[:], xT[:], waug_t[:], start=True, stop=True)

            stage = stpool.tile([P, cfg.hext_w], BF16, tag="stage")
            nc.vector.memset(stage[:, hc + 16 :], 0.0)
            nc.vector.tensor_copy(stage[:, 0:hc], ph[:, 0:hc])
            nc.vector.tensor_copy(
                stage[:, hc : hc + 16].bitcast(F32), ph[:, hc:naug]
            )
            nc.sync.dma_start(hext[n0 : n0 + P, :], stage[:])

        # dummy hext rows for padded edge slots: a_dst = -1000 => ex == 0
        dstage = stpool.tile([P, cfg.hext_w], BF16, tag="stage")
        nc.vector.memset(dstage[:], 0.0)
        nc.vector.memset(dstage[:, hc : hc + 16].bitcast(F32), -1000.0)
        nc.sync.dma_start(hext[cfg.n_pad : cfg.n_pad + P, :], dstage[:])

        nc.leave_named_scope("h_phase", h_scope, False)
        tc.strict_bb_all_engine_barrier()

        # ---------------- phase 2: edge message passing ----------------
        e_scope = nc.enter_named_scope("edge_phase", False)[0]

        max_ch = max(chunks)
        hpool = ctx.enter_context(tc.tile_pool(name="hrow", bufs=3))
        apool = ctx.enter_context(tc.tile_pool(name="arow", bufs=3))
        mpool = ctx.enter_context(tc.tile_pool(name="msg", bufs=3))
        ohpool = ctx.enter_context(tc.tile_pool(name="oh", bufs=3))
        spool = ctx.enter_context(tc.tile_pool(name="small", bufs=3))
        epool = ctx.enter_context(tc.tile_pool(name="exf", bufs=3))
        gpool = ctx.enter_context(tc.tile_pool(name="gidx", bufs=3))
        ps_e = ctx.enter_context(tc.tile_pool(name="ps_e", bufs=4, space="PSUM"))

        # Split gathers: SWDGE descriptor-ring carveout holds ~256 descs per
        # partition; one gather emits num_idxs/16 descs per partition, so keep
        # each call at <= GMAX indices.
        GMAX_CH = 8  # 1024 indices / call
        sub_lens = set()
        for nch in set(chunks):
            for c0 in range(0, nch, GMAX_CH):
                sub_lens.add(min(GMAX_CH, nch - c0) * P)
        lregs = {l: nc.gpsimd.to_reg(l) for l in sorted(sub_lens)}

        def next_q():
            # queue_num is rewritten post-scheduling (see below) to match the
            # DMASW sem lane Tile assigned; sem lanes can't span queues.
            return 0

        off = 0
        chunk_base = 0
        for m in range(cfg.n_tiles):
            nch = chunks[m]
            L16 = nch * P // 16
            g1t = gpool.tile([P, (max_ch * P) // 16], I16, tag="g1t")
            nc.sync.dma_start(g1t[:, 0:L16], g1[:, off // 16 : off // 16 + L16])
            g2t = gpool.tile([P, (max_ch * P) // 16], I16, tag="g2t")
            nc.sync.dma_start(g2t[:, 0:L16], g2[:, off // 16 : off // 16 + L16])
            hrow = hpool.tile([P, max_ch, cfg.hext_w], BF16)
            arow = apool.tile([P, max_ch, cfg.aux_w], BF16)
            for c0 in range(0, nch, GMAX_CH):
                cc = min(GMAX_CH, nch - c0)
                ll = cc * P
                o0 = c0 * P
                nc.gpsimd.dma_gather(
                    hrow[:, c0 : c0 + cc, :],
                    hext[:, :],
                    g1t[:, o0 // 16 : (o0 + ll) // 16],
                    ll,
                    lregs[ll],
                    cfg.hext_w,
                    queue_num=next_q(),
                )
                nc.gpsimd.dma_gather(
                    arow[:, c0 : c0 + cc, :],
                    hext[:, hc : hc + P],
                    g2t[:, o0 // 16 : (o0 + ll) // 16],
                    ll,
                    lregs[ll],
                    cfg.aux_w,
                    elem_step=cfg.hext_w,
                    queue_num=next_q(),
                )
            off += nch * P

            # alpha = leaky_relu(a_src[src] + a_dst[dst]); ex = exp(alpha)
            alpha = spool.tile([P, max_ch, heads], F32, tag="alpha")
            nc.vector.tensor_add(
                alpha[:, 0:nch, :],
                hrow[:, 0:nch, hc : hc + 8].bitcast(F32),
                arow[:, 0:nch, 8:16].bitcast(F32),
            )
            lrt = spool.tile([P, max_ch, heads], F32, tag="lrt")
            nc.vector.tensor_scalar_mul(lrt[:, 0:nch, :], alpha[:, 0:nch, :], cfg.neg_slope)
            nc.vector.tensor_max(alpha[:, 0:nch, :], alpha[:, 0:nch, :], lrt[:, 0:nch, :])
            # exp evaluated on the (otherwise idle) scalar engine directly in
            # broadcast-expanded (c, h) form: one op yields ex for every
            # channel. The DVE multiply below then has contiguous operands
            # (2x mode) instead of a step-0 broadcast.
            exf = epool.tile([P, max_ch, hc], BF16, tag="exf")
            nc.scalar.activation(
                exf[:, 0:nch, :].rearrange("p n (c h) -> p n c h", h=heads),
                alpha[:, 0:nch, :]
                .rearrange("p n h -> p n () h")
                .broadcast_to((P, nch, cfg.d_model, heads)),
                mybir.ActivationFunctionType.Exp,
            )

            msg = mpool.tile([P, max_ch, cfg.mm_w], BF16)
            nc.vector.tensor_tensor(
                msg[:, 0:nch, 0:hc],
                hrow[:, 0:nch, 0:hc],
                exf[:, 0:nch, :],
                op=mybir.AluOpType.mult,
            )
            nc.vector.tensor_copy(
                msg[:, 0:nch, hc : cfg.mm_w], exf[:, 0:nch, 0:heads]
            )

            # one-hot(dst_local) for all chunks of the tile in one DVE op
            oh_all = ohpool.tile([P, max_ch, P], BF16)
            nc.vector.tensor_tensor(
                oh_all[:, 0:nch, :],
                t2_t[:].rearrange("p d -> p () d").broadcast_to((P, nch, P)),
                dls[:, chunk_base : chunk_base + nch]
                .rearrange("p n -> p n ()")
                .broadcast_to((P, nch, P)),
                op=mybir.AluOpType.is_equal,
            )

            # segment sums via one-hot matmul into PSUM
            pe = ps_e.tile([P, cfg.mm_w], F32)
            for ch in range(nch):
                nc.tensor.matmul(
                    pe[:],
                    oh_all[:, ch, :],
                    msg[:, ch, :],
                    start=(ch == 0),
                    stop=(ch == nch - 1),
                )

            # out = (numerator / denom).mean(heads) + bias
            r = spool.tile([P, heads], F32, tag="r")
            nc.vector.reciprocal(r[:], pe[:, hc : cfg.mm_w])
            nc.vector.tensor_scalar_mul(r[:], r[:], 1.0 / heads)
            wm = spool.tile([P, cfg.d_model, heads], F32, tag="wm")
            nc.vector.tensor_tensor(
                wm[:],
                pe[:, 0:hc].rearrange("p (c h) -> p c h", h=heads),
                r[:].rearrange("p h -> p () h").broadcast_to((P, cfg.d_model, heads)),
                op=mybir.AluOpType.mult,
            )
            onode = spool.tile([P, cfg.d_model], F32, tag="onode")
            nc.vector.tensor_reduce(
                onode[:],
                wm[:],
                axis=mybir.AxisListType.X,
                op=mybir.AluOpType.add,
            )
            nc.vector.tensor_add(onode[:], onode[:], bias_t[:])
            n0 = m * P
            nrows = min(P, cfg.n_nodes - n0)
            nc.sync.dma_start(out[n0 : n0 + nrows, :], onode[:nrows, :])
            chunk_base += nch

        nc.leave_named_scope("edge_phase", e_scope, False)

    # Spread gathers over the 4 SWDGE queues. Each DMASW sem lane is locked to
    # one queue, so derive the queue from the lane Tile assigned (k % 4).
    import re

    for f in nc.m.functions:
        for bb in f.blocks:
            for inst in bb.instructions:
                if isinstance(inst, mybir.InstDMAGatherAnt):
                    si = inst.sync_info
                    if si and si.on_update:
                        name = getattr(si.on_update[0], "ant_name", "") or ""
                        mt = re.match(r"DMASW(\d+)", name)
                        if mt:
                            inst.queue_num = int(mt.group(1)) % 4

    nc.compile()
    return nc


_CACHE = {}


def _prepare(x, edge_index, W, att_src, att_dst, bias):
    cfg = CFG
    x = np.asarray(x, np.float32)
    key = hash(np.asarray(edge_index).tobytes())
    if key not in _CACHE:
        g1w, g2w, chunks, dl_all = preprocess_edges(cfg, edge_index)
        nc = build_nc(cfg, chunks, g1w.shape[1], debug=False, num_devices=N_CORES)
        _CACHE.clear()
        _CACHE[key] = (nc, g1w, g2w, dl_all)
    nc, g1w, g2w, dl_all = _CACHE[key]
    consts = build_consts(cfg, W, att_src, att_dst, bias)
    in_maps = []
    for t in range(T_STEPS):
        in_maps.append(
            {
                "xt": np.ascontiguousarray(x[:, t, :]),
                "g1": g1w,
                "g2": g2w,
                "dl": dl_all,
                **consts,
            }
        )
    return nc, in_maps


def kernel(x, edge_index, W, att_src, att_dst, bias):
    nc, in_maps = _prepare(x, edge_index, W, att_src, att_dst, bias)
    res = run_bass_kernel_spmd(nc, in_maps, core_ids=list(range(N_CORES)))
    outs = [res.results[t]["out"] for t in range(T_STEPS)]
    return np.stack(outs, axis=1)  # [N, T, C]


def kernel_profiled(x, edge_index, W, att_src, att_dst, bias):
    """Run with NTFF tracing; returns (output, exec_time_ns, results obj)."""
    nc, in_maps = _prepare(x, edge_index, W, att_src, att_dst, bias)
    res = run_bass_kernel_spmd(
        nc, in_maps, core_ids=list(range(N_CORES)), trace=True
    )
    outs = [res.results[t]["out"] for t in range(T_STEPS)]
    return np.stack(outs, axis=1), res.exec_time_ns, res



# revision 18
# speedup vs baseline: 1.2109x; 1.2109x over previous
"""Self-contained Trainium2 Bass kernel for a batched (time-stepped) GAT layer.

Problem: x [N=20000, T=8, F=128], edge_index [2, E=320000] (+self loops),
W [128, 256] (4 heads x 64), att_src/att_dst [4, 64], bias [64].
Per time step: GATConv (concat=False -> head mean) with softmax attention.
Output: [N, T, 64] f32.

Sharding (8 cores): 2 step-quads x 4 node-quarters. Each core handles 4 time
steps for ~5000 destination nodes. The per-edge h[src] gather row packs all 4
steps (2304B), so gather descriptor count (the gpsimd/SWDGE bottleneck) drops
4x vs one-step rows, and every per-edge vector op is batched across steps.

Per-core algorithm:
  Phase 1 (dense, all 157 node tiles x 4 steps): h_s = x_s @ W_aug where W_aug
    also yields per-node a_src/a_dst logits. Rows written to HBM 'hext'
    [n, 1152 bf16]: 4x256 h (c,h)-major | 4x4 a_src f32 | 4x4 a_dst f32 | pad.
  Phase 2 (edges of our quarter, sorted by destination, per 128-dst tile,
    sliced into 8-chunk pieces):
    - dma_gather hext rows by src (2304B)     -> h[src], a_src[src]
    - dma_gather hext tail 256B slices by dst -> a_dst[dst]
    - alpha = leaky_relu(a_src+a_dst) for 4 steps x 4 heads in 2 DVE ops
    - ex = exp(alpha) broadcast-expanded on the scalar engine to (s, c, h)
    - msg = h * ex (one DVE op over all 4 steps, 2x bf16 mode)
    - one-hot(dst_local) matmuls accumulate per-step segment sums in PSUM:
      numerator [128, 256] + denominator [128, 4] per step
    - batched epilogue every 4 tiles: out = (num/den).mean(heads) + bias
"""

import numpy as np
import ml_dtypes
from contextlib import ExitStack

import concourse.bass as bass
import concourse.bacc as bacc
import concourse.mybir as mybir
import concourse.tile as tile
from concourse import library_config
from concourse.bass_utils import run_bass_kernel_spmd

F32 = mybir.dt.float32
BF16 = mybir.dt.bfloat16
I16 = mybir.dt.int16

P = 128
N_NODES = 20000
IN_DIM = 128
HEADS = 4
D_MODEL = 64
HC = HEADS * D_MODEL          # 256
T_STEPS = 8
S = 4                         # time steps packed per core
NEG_SLOPE = 0.2
N_CORES = 8

N_TILES = (N_NODES + P - 1) // P          # 157
N_PAD = N_TILES * P                        # 20096
DUMMY_ROW = N_PAD                          # a_dst = -1000 -> ex == 0
HEXT_ROWS = N_PAD + P
# bf16 cols: 4*256 h | 32 (4x4 a_src f32) | 32 (4x4 a_dst f32) | pad
HEXT_W = S * HC + P                        # 1152 cols = 2304 B
AUX_OFF = S * HC                           # 1024 (bf16 col of a_src block)
AUX_W = P                                  # 256B tail slice for the dst gather
MM_W = HC + HEADS                          # 260 matmul rhs width per step

Q_TILES = 40                               # tiles per quarter (ghost-padded)
QT_BOUNDS = [0, 40, 79, 118, 157]          # quarter tile boundaries
QN_BOUNDS = [0, 5120, 10112, 15104, 20000]  # quarter node boundaries
OUT_ROWS = Q_TILES * P                     # 5120 rows per core (tail = scratch)
SL = 8                                     # chunks per gather slice
EPI_G = 4                                  # tiles per batched epilogue group


def preprocess_edges(edge_index):
    """Sort (edges + self loops) by destination; build per-quarter gather
    indices with tile shapes equalized across quarters (SPMD: all cores run
    the identical program; only the index *contents* differ per core).

    Returns (nch: [Q_TILES] chunks per local tile, per_quarter: list of
    (gidx [128, sum_nch*16] int16, dl [128, sum_nch] bf16)).
    """
    loops = np.arange(N_NODES, dtype=np.int64)
    src = np.concatenate([np.asarray(edge_index[0], dtype=np.int64), loops])
    dst = np.concatenate([np.asarray(edge_index[1], dtype=np.int64), loops])
    order = np.argsort(dst, kind="stable")
    src_s = src[order]
    dst_s = dst[order]
    counts = np.bincount(dst_s // P, minlength=N_TILES)
    starts = np.concatenate([[0], np.cumsum(counts)])

    # equalized chunks per local tile index
    nch = np.ones(Q_TILES, np.int64)
    for q in range(4):
        for j in range(QT_BOUNDS[q + 1] - QT_BOUNDS[q]):
            g = QT_BOUNDS[q] + j
            nch[j] = max(nch[j], (counts[g] + P - 1) // P)

    def wrap(flat):
        w = flat.reshape(-1, 16).T.copy()
        return np.tile(w, (8, 1)).copy()

    per_quarter = []
    for q in range(4):
        g1_parts, g2_parts, dl_parts = [], [], []
        for j in range(Q_TILES):
            g = QT_BOUNDS[q] + j
            lpad = int(nch[j]) * P
            g1 = np.zeros(lpad, np.int16)
            g2 = np.full(lpad, DUMMY_ROW, np.int16)
            dl = np.full(lpad, 200.0, np.float64)
            if g < QT_BOUNDS[q + 1]:
                length = int(counts[g])
                pos = int(starts[g])
                g1[:length] = src_s[pos : pos + length]
                g2[:length] = dst_s[pos : pos + length]
                dl[:length] = (dst_s[pos : pos + length] - g * P).astype(np.float64)
            g1_parts.append(wrap(g1))
            g2_parts.append(wrap(g2))
            dl_parts.append(dl.reshape(-1, P).T.astype(ml_dtypes.bfloat16))
        # per tile: [g1 | g2] so one resident idx tensor serves both gathers
        gidx = np.concatenate(
            [np.concatenate([a, b], axis=1) for a, b in zip(g1_parts, g2_parts)],
            axis=1,
        )
        dl_all = np.concatenate(dl_parts, axis=1).copy()
        per_quarter.append((np.ascontiguousarray(gidx), np.ascontiguousarray(dl_all)))
    return nch.tolist(), per_quarter


def build_consts(W, att_src, att_dst, bias):
    W = np.asarray(W, np.float32)
    att_src = np.asarray(att_src, np.float32)
    att_dst = np.asarray(att_dst, np.float32)
    bias = np.asarray(bias, np.float32)
    Wr = W.reshape(IN_DIM, HEADS, D_MODEL)
    a_src_cols = np.einsum("fhc,hc->fh", Wr, att_src)
    a_dst_cols = np.einsum("fhc,hc->fh", Wr, att_dst)
    # h channels stay (h, c)-major (natural W layout): col = h*D + c, so the
    # per-head epilogue reduce is a pair of half-width slice adds.
    waug = np.concatenate([W, a_src_cols, a_dst_cols], axis=1)
    biasrep = np.tile(bias[None, :], (P, 1)).astype(np.float32)
    t2row = np.tile(np.arange(P, dtype=ml_dtypes.bfloat16)[None, :], (P, 1)).copy()
    return {
        "waug": np.ascontiguousarray(waug, np.float32),
        "biasrep": biasrep,
        "t2row": t2row,
    }


def build_nc(nch, debug=False, num_devices=N_CORES):
    """Build the SPMD Bass program (identical across cores)."""
    nc = bacc.Bacc(
        "TRN2",
        target_bir_lowering=False,
        debug=debug,
        num_devices=num_devices,
        num_swdge_queues=4,
    )
    sum_nch = sum(nch)
    max_nch = max(nch)
    naug = HC + 2 * HEADS  # 264

    xt = nc.dram_tensor("xt", [P, S, N_PAD], BF16, kind="ExternalInput")
    waug = nc.dram_tensor("waug", [IN_DIM, naug], F32, kind="ExternalInput")
    biasrep = nc.dram_tensor("biasrep", [P, D_MODEL], F32, kind="ExternalInput")
    t2row = nc.dram_tensor("t2row", [P, P], BF16, kind="ExternalInput")
    dl = nc.dram_tensor("dl", [P, sum_nch], BF16, kind="ExternalInput")
    gidx = nc.dram_tensor("gidx", [P, sum_nch * 16], I16, kind="ExternalInput")
    hext = nc.dram_tensor("hext", [HEXT_ROWS, HEXT_W], BF16, kind="Internal")
    out = nc.dram_tensor("out", [OUT_ROWS, S, D_MODEL], F32, kind="ExternalOutput")

    with tile.TileContext(nc) as tc, ExitStack() as ctx:
        nc.gpsimd.load_library(library_config.mlp)
        tc.no_sync_barrier()

        consts = ctx.enter_context(tc.tile_pool(name="consts", bufs=1))
        waug_f32 = consts.tile([P, naug], F32)
        nc.sync.dma_start(waug_f32[:], waug[:, :])
        waug_t = consts.tile([P, naug], BF16)
        nc.vector.tensor_copy(waug_t[:], waug_f32[:])
        bias_t = consts.tile([P, D_MODEL], F32)
        nc.sync.dma_start(bias_t[:], biasrep[:, :])
        t2_t = consts.tile([P, P], BF16)
        nc.sync.dma_start(t2_t[:], t2row[:, :])
        dls = consts.tile([P, sum_nch], BF16)
        nc.sync.dma_start(dls[:], dl[:, :])

        # ---------------- phase 1: dense h + logits, all nodes x 4 steps ----
        h_scope = nc.enter_named_scope("h_phase", False)[0]
        with ExitStack() as p1:
            XG = 8  # node tiles per x load
            xpool = p1.enter_context(tc.tile_pool(name="x", bufs=2))
            stpool = p1.enter_context(tc.tile_pool(name="stage", bufs=3))
            ps1 = p1.enter_context(tc.tile_pool(name="ps1", bufs=2, space="PSUM"))

            for g0 in range(0, N_TILES, XG):
                gt = min(XG, N_TILES - g0)
                xg = xpool.tile([P, S, XG * P], BF16, tag="xg")
                nc.sync.dma_start(
                    xg[:, :, 0 : gt * P], xt[:, :, g0 * P : (g0 + gt) * P]
                )
                for t in range(gt):
                    m = g0 + t
                    ph = ps1.tile([P, S, 512], F32)
                    for s in range(S):
                        nc.tensor.matmul(
                            ph[:, s, 0:naug],
                            xg[:, s, t * P : (t + 1) * P],
                            waug_t[:],
                            start=True,
                            stop=True,
                        )
                    stage = stpool.tile([P, HEXT_W], BF16, tag="stage")
                    nc.vector.tensor_copy(
                        stage[:, 0 : S * HC].rearrange("p (s c) -> p s c", s=S),
                        ph[:, :, 0:HC],
                    )
                    # aux layout: 16 f32 a_src (s-major) | 16 f32 a_dst
                    nc.vector.tensor_copy(
                        stage[:, AUX_OFF : AUX_OFF + 32]
                        .bitcast(F32)
                        .rearrange("p (s v) -> p s v", s=S),
                        ph[:, :, HC : HC + HEADS],
                    )
                    nc.vector.tensor_copy(
                        stage[:, AUX_OFF + 32 : AUX_OFF + 64]
                        .bitcast(F32)
                        .rearrange("p (s v) -> p s v", s=S),
                        ph[:, :, HC + HEADS : naug],
                    )
                    nc.sync.dma_start(hext[m * P : (m + 1) * P, :], stage[:])

            # dummy row for padded edge slots: a_src/a_dst = -1000 => ex == 0
            dstage = stpool.tile([P, HEXT_W], BF16, tag="stage")
            nc.vector.memset(dstage[:], 0.0)
            nc.vector.memset(
                dstage[:, AUX_OFF : AUX_OFF + 64].bitcast(F32), -1000.0
            )
            nc.sync.dma_start(hext[N_PAD : N_PAD + P, :], dstage[:])

        nc.leave_named_scope("h_phase", h_scope, False)
        tc.strict_bb_all_engine_barrier()

        # ---------------- phase 2: edge message passing ---------------------
        e_scope = nc.enter_named_scope("edge_phase", False)[0]

        hgp = ctx.enter_context(tc.tile_pool(name="hg", bufs=2))
        gixp = ctx.enter_context(tc.tile_pool(name="gix", bufs=2))
        arp = ctx.enter_context(tc.tile_pool(name="ar", bufs=2))
        exp_ = ctx.enter_context(tc.tile_pool(name="exf", bufs=2))
        msgp = ctx.enter_context(tc.tile_pool(name="msg", bufs=2))
        alp = ctx.enter_context(tc.tile_pool(name="al", bufs=3))
        ohp = ctx.enter_context(tc.tile_pool(name="oh", bufs=2))
        pep = ctx.enter_context(tc.tile_pool(name="pe", bufs=2, space="PSUM"))
        nump = ctx.enter_context(tc.tile_pool(name="num", bufs=2))
        wmp = ctx.enter_context(tc.tile_pool(name="wm", bufs=1))
        onp = ctx.enter_context(tc.tile_pool(name="on", bufs=2))

        lregs = {}
        for j in range(Q_TILES):
            for v in (min(SL, nch[j] - c0) * P for c0 in range(0, nch[j], SL)):
                lregs.setdefault(v, None)
            lregs.setdefault(nch[j] * P, None)
        for v in sorted(lregs):
            lregs[v] = nc.gpsimd.to_reg(v)

        chunk_base = 0
        num_t = None
        for j in range(Q_TILES):
            nj = nch[j]
            i0 = chunk_base * 16  # idx col offset (g1 at i0, g2 at i0+nj*8)

            gix = gixp.tile([P, max_nch * 16], I16)
            nc.sync.dma_start(gix[:, 0 : nj * 16], gidx[:, i0 : i0 + nj * 16])
            arow = arp.tile([P, max_nch, AUX_W], BF16)
            # <=1024 idxs per gather call (SWDGE descriptor ring capacity)
            for c0 in range(0, nj, SL):
                cc = min(SL, nj - c0)
                nc.gpsimd.dma_gather(
                    arow[:, c0 : c0 + cc, :],
                    hext[:, AUX_OFF : AUX_OFF + AUX_W],
                    gix[:, nj * 8 + c0 * 8 : nj * 8 + (c0 + cc) * 8],
                    cc * P,
                    lregs[cc * P],
                    AUX_W,
                    elem_step=HEXT_W,
                    queue_num=0,
                )

            pe = pep.tile([P, S, 512], F32)
            first = True
            for c0 in range(0, nj, SL):
                cc = min(SL, nj - c0)
                ll = cc * P
                hg = hgp.tile([P, SL, HEXT_W], BF16)
                nc.gpsimd.dma_gather(
                    hg[:, 0:cc, :],
                    hext[:, :],
                    gix[:, c0 * 8 : (c0 + cc) * 8],
                    ll,
                    lregs[ll],
                    HEXT_W,
                    queue_num=0,
                )

                # aux block per row: 16 f32 a_src (s-major) | 16 f32 a_dst
                alpha = alp.tile([P, SL, S * HEADS], F32, tag="alpha")
                nc.vector.tensor_add(
                    alpha[:, 0:cc, :],
                    hg[:, 0:cc, AUX_OFF : AUX_OFF + 32].bitcast(F32),
                    arow[:, c0 : c0 + cc, 32:64].bitcast(F32),
                )
                lr = alp.tile([P, SL, S * HEADS], F32, tag="lr")
                nc.vector.scalar_tensor_tensor(
                    lr[:, 0:cc, :],
                    alpha[:, 0:cc, :],
                    NEG_SLOPE,
                    alpha[:, 0:cc, :],
                    op0=mybir.AluOpType.mult,
                    op1=mybir.AluOpType.max,
                )
                exf = exp_.tile([P, SL, S, HC], BF16, tag="exf")
                nc.scalar.activation(
                    exf[:, 0:cc, :, :].rearrange(
                        "p n s (h c) -> p (n s) h c", h=HEADS
                    ),
                    lr[:, 0:cc, :]
                    .rearrange("p n (s h) -> p (n s) h ()", s=S)
                    .broadcast_to((P, cc * S, HEADS, D_MODEL)),
                    mybir.ActivationFunctionType.Exp,
                )
                msg = msgp.tile([P, SL, S, MM_W], BF16)
                nc.vector.tensor_tensor(
                    msg[:, 0:cc, :, 0:HC].rearrange("p n s c -> p (n s) c"),
                    hg[:, 0:cc, 0 : S * HC],
                    exf[:, 0:cc, :, :].rearrange("p n s c -> p (n s) c"),
                    op=mybir.AluOpType.mult,
                )
                nc.vector.tensor_copy(
                    msg[:, 0:cc, :, HC:MM_W].rearrange("p n s h -> p (n s) h"),
                    exf[:, 0:cc, :, :].rearrange(
                        "p n s (h c) -> p (n s) h c", h=HEADS
                    )[:, :, :, 0:1],
                )
                oh = ohp.tile([P, SL, P], BF16)
                nc.vector.tensor_tensor(
                    oh[:, 0:cc, :],
                    t2_t[:].rearrange("p d -> p () d").broadcast_to((P, cc, P)),
                    dls[:, chunk_base + c0 : chunk_base + c0 + cc]
                    .rearrange("p n -> p n ()")
                    .broadcast_to((P, cc, P)),
                    op=mybir.AluOpType.is_equal,
                )
                for ch in range(cc):
                    last = c0 + ch == nj - 1
                    for s in range(S):
                        nc.tensor.matmul(
                            pe[:, s, 0:MM_W],
                            oh[:, ch, :],
                            msg[:, ch, s, :],
                            start=first,
                            stop=last,
                        )
                    first = False

            g = j % EPI_G
            if g == 0:
                num_t = nump.tile([P, EPI_G, S, MM_W], F32)
            nc.vector.tensor_copy(num_t[:, g, :, :], pe[:, :, 0:MM_W])

            if g == EPI_G - 1:
                j0 = j - (EPI_G - 1)
                r = alp.tile([P, EPI_G, S, HEADS], F32, tag="r")
                nc.vector.reciprocal(r[:], num_t[:, :, :, HC:MM_W])
                # broadcast-expand (1/den)/4 per head on the scalar engine
                rexp = wmp.tile([P, EPI_G, S, HC], F32, tag="rexp")
                nc.scalar.activation(
                    rexp[:].rearrange("p g s (h c) -> p (g s) h c", h=HEADS),
                    r[:]
                    .rearrange("p g s h -> p (g s) h ()")
                    .broadcast_to((P, EPI_G * S, HEADS, D_MODEL)),
                    mybir.ActivationFunctionType.Copy,
                    scale=1.0 / HEADS,
                )
                wm = num_t[:, :, :, 0:HC]  # in-place: num *= rexp
                nc.vector.tensor_tensor(
                    wm, wm, rexp[:], op=mybir.AluOpType.mult
                )
                # head mean: (h,c)-major -> two half-width slice adds
                th = wmp.tile([P, EPI_G, S, 2 * D_MODEL], F32, tag="th")
                nc.vector.tensor_add(
                    th[:],
                    wm[:, :, :, 0 : 2 * D_MODEL],
                    wm[:, :, :, 2 * D_MODEL : 4 * D_MODEL],
                )
                onode = onp.tile([P, EPI_G, S, D_MODEL], F32, tag="onode")
                nc.vector.tensor_add(
                    onode[:], th[:, :, :, 0:D_MODEL], th[:, :, :, D_MODEL:]
                )
                nc.vector.tensor_tensor(
                    onode[:].rearrange("p g s c -> p (g s) c"),
                    onode[:].rearrange("p g s c -> p (g s) c"),
                    bias_t[:]
                    .rearrange("p c -> p () c")
                    .broadcast_to((P, EPI_G * S, D_MODEL)),
                    op=mybir.AluOpType.add,
                )
                nc.sync.dma_start(
                    out[j0 * P : (j + 1) * P, :, :].rearrange(
                        "(g p) s c -> p g (s c)", p=P
                    ),
                    onode[:].rearrange("p g s c -> p g (s c)"),
                )
            chunk_base += nj

        nc.leave_named_scope("edge_phase", e_scope, False)

    # Spread gathers over the 4 SWDGE queues. Each DMASW sem lane is locked to
    # one queue, so derive the queue from the lane Tile assigned (k % 4).
    import re

    for f in nc.m.functions:
        for bb in f.blocks:
            for inst in bb.instructions:
                if isinstance(inst, mybir.InstDMAGatherAnt):
                    si = inst.sync_info
                    if si and si.on_update:
                        name = getattr(si.on_update[0], "ant_name", "") or ""
                        mt = re.match(r"DMASW(\d+)", name)
                        if mt:
                            inst.queue_num = int(mt.group(1)) % 4

    nc.compile()
    return nc


_CACHE = {}


def _prepare(x, edge_index, W, att_src, att_dst, bias):
    x = np.asarray(x, np.float32)
    key = hash(np.asarray(edge_index).tobytes())
    if key not in _CACHE:
        nch, per_quarter = preprocess_edges(edge_index)
        nc = build_nc(nch, debug=False, num_devices=N_CORES)
        _CACHE.clear()
        _CACHE[key] = (nc, nch, per_quarter)
    nc, nch, per_quarter = _CACHE[key]
    consts = build_consts(W, att_src, att_dst, bias)
    # x [N, T, F] f32 -> per step-quad [F, S, N_PAD] bf16
    xq = []
    for t0 in (0, S):
        xp = np.zeros((IN_DIM, S, N_PAD), ml_dtypes.bfloat16)
        xp[:, :, 0:N_NODES] = x[:, t0 : t0 + S, :].transpose(2, 1, 0).astype(
            ml_dtypes.bfloat16
        )
        xq.append(np.ascontiguousarray(xp))
    in_maps = []
    for c in range(N_CORES):
        q = c % 4
        gidx, dl_all = per_quarter[q]
        in_maps.append(
            {
                "xt": xq[c // 4],
                "gidx": gidx,
                "dl": dl_all,
                **consts,
            }
        )
    return nc, in_maps


def _assemble(res):
    out = np.empty((N_NODES, T_STEPS, D_MODEL), np.float32)
    for c in range(N_CORES):
        q = c % 4
        t0 = (c // 4) * S
        n0, n1 = QN_BOUNDS[q], QN_BOUNDS[q + 1]
        core_out = res.results[c]["out"]  # [OUT_ROWS, S, 64]
        out[n0:n1, t0 : t0 + S, :] = core_out[0 : n1 - n0]
    return out


def kernel(x, edge_index, W, att_src, att_dst, bias):
    nc, in_maps = _prepare(x, edge_index, W, att_src, att_dst, bias)
    res = run_bass_kernel_spmd(nc, in_maps, core_ids=list(range(N_CORES)))
    return _assemble(res)


def kernel_profiled(x, edge_index, W, att_src, att_dst, bias):
    """Run with NTFF tracing; returns (output, exec_time_ns, results obj)."""
    nc, in_maps = _prepare(x, edge_index, W, att_src, att_dst, bias)
    res = run_bass_kernel_spmd(
        nc, in_maps, core_ids=list(range(N_CORES)), trace=True
    )
    return _assemble(res), res.exec_time_ns, res


# revision 22
# speedup vs baseline: 1.5796x; 1.3045x over previous
"""Self-contained Trainium2 Bass kernel for a batched (time-stepped) GAT layer.

Problem: x [N=20000, T=8, F=128], edge_index [2, E=320000] (+self loops),
W [128, 256] (4 heads x 64), att_src/att_dst [4, 64], bias [64].
Per time step: GATConv (concat=False -> head mean) with softmax attention.
Output: [N, T, 64] f32.

Sharding (8 cores): 2 step-quads x 4 node-quarters. Each core handles 4 time
steps for ~5000 destination nodes. The per-edge h[src] gather row packs all 4
steps (2304B), so gather descriptor count (the gpsimd/SWDGE bottleneck) drops
4x vs one-step rows, and every per-edge vector op is batched across steps.

Per-core algorithm:
  Phase 1 (dense, all 157 node tiles x 4 steps): h_s = x_s @ W_aug where W_aug
    also yields per-node a_src/a_dst logits. Rows written to HBM 'hext'
    [n, 1152 bf16]: 4x256 h (c,h)-major | 4x4 a_src f32 | 4x4 a_dst f32 | pad.
  Phase 2 (edges of our quarter, sorted by destination, per 128-dst tile,
    sliced into 8-chunk pieces):
    - dma_gather hext rows by src (2304B)     -> h[src], a_src[src]
    - dma_gather hext tail 256B slices by dst -> a_dst[dst]
    - alpha = leaky_relu(a_src+a_dst) for 4 steps x 4 heads in 2 DVE ops
    - ex = exp(alpha) broadcast-expanded on the scalar engine to (s, c, h)
    - msg = h * ex (one DVE op over all 4 steps, 2x bf16 mode)
    - one-hot(dst_local) matmuls accumulate per-step segment sums in PSUM:
      numerator [128, 256] + denominator [128, 4] per step
    - batched epilogue every 4 tiles: out = (num/den).mean(heads) + bias
"""

import numpy as np
import ml_dtypes
from contextlib import ExitStack

import concourse.bass as bass
import concourse.bacc as bacc
import concourse.mybir as mybir
import concourse.tile as tile
from concourse import library_config
from concourse.bass_utils import run_bass_kernel_spmd

F32 = mybir.dt.float32
BF16 = mybir.dt.bfloat16
I16 = mybir.dt.int16

P = 128
N_NODES = 20000
IN_DIM = 128
HEADS = 4
D_MODEL = 64
HC = HEADS * D_MODEL          # 256
T_STEPS = 8
S = 4                         # time steps packed per core
NEG_SLOPE = 0.2
N_CORES = 8

N_TILES = (N_NODES + P - 1) // P          # 157
N_PAD = N_TILES * P                        # 20096
DUMMY_ROW = N_PAD                          # a_dst = -1000 -> ex == 0
HEXT_ROWS = N_PAD + P
# bf16 cols: 4*256 h | 32 (4x4 a_src f32) | 32 (4x4 a_dst f32) | pad
HEXT_W = S * HC + P                        # 1152 cols = 2304 B
AUX_OFF = S * HC                           # 1024 (bf16 col of a_src block)
AUX_W = P                                  # 256B tail slice for the dst gather
MM_W = HC + HEADS                          # 260 matmul rhs width per step

Q_TILES = 40                               # tiles per quarter (ghost-padded)
QT_BOUNDS = [0, 40, 79, 118, 157]          # quarter tile boundaries
QN_BOUNDS = [0, 5120, 10112, 15104, 20000]  # quarter node boundaries
OUT_ROWS = Q_TILES * P                     # 5120 rows per core (tail = scratch)
SL = 4                                     # chunks per gather slice
EPI_G = 4                                  # tiles per batched epilogue group


def preprocess_edges(edge_index):
    """Sort (edges + self loops) by destination; build per-quarter gather
    indices with tile shapes equalized across quarters (SPMD: all cores run
    the identical program; only the index *contents* differ per core).

    Returns (nch: [Q_TILES] chunks per local tile, per_quarter: list of
    (gidx [128, sum_nch*16] int16, dl [128, sum_nch] bf16)).
    """
    loops = np.arange(N_NODES, dtype=np.int64)
    src = np.concatenate([np.asarray(edge_index[0], dtype=np.int64), loops])
    dst = np.concatenate([np.asarray(edge_index[1], dtype=np.int64), loops])
    order = np.argsort(dst, kind="stable")
    src_s = src[order]
    dst_s = dst[order]
    counts = np.bincount(dst_s // P, minlength=N_TILES)
    starts = np.concatenate([[0], np.cumsum(counts)])

    # equalized chunks per local tile index
    nch = np.ones(Q_TILES, np.int64)
    for q in range(4):
        for j in range(QT_BOUNDS[q + 1] - QT_BOUNDS[q]):
            g = QT_BOUNDS[q] + j
            nch[j] = max(nch[j], (counts[g] + P - 1) // P)

    def wrap(flat):
        w = flat.reshape(-1, 16).T.copy()
        return np.tile(w, (8, 1)).copy()

    per_quarter = []
    for q in range(4):
        g1_parts, g2_parts, dl_parts = [], [], []
        for j in range(Q_TILES):
            g = QT_BOUNDS[q] + j
            lpad = int(nch[j]) * P
            g1 = np.zeros(lpad, np.int16)
            g2 = np.full(lpad, DUMMY_ROW, np.int16)
            dl = np.full(lpad, 200.0, np.float64)
            if g < QT_BOUNDS[q + 1]:
                length = int(counts[g])
                pos = int(starts[g])
                g1[:length] = src_s[pos : pos + length]
                g2[:length] = dst_s[pos : pos + length]
                dl[:length] = (dst_s[pos : pos + length] - g * P).astype(np.float64)
            g1_parts.append(wrap(g1))
            g2_parts.append(wrap(g2))
            dl_parts.append(dl.reshape(-1, P).T.astype(ml_dtypes.bfloat16))
        # per tile: [g1 | g2] so one resident idx tensor serves both gathers
        gidx = np.concatenate(
            [np.concatenate([a, b], axis=1) for a, b in zip(g1_parts, g2_parts)],
            axis=1,
        )
        dl_all = np.concatenate(dl_parts, axis=1).copy()
        per_quarter.append((np.ascontiguousarray(gidx), np.ascontiguousarray(dl_all)))
    return nch.tolist(), per_quarter


def build_consts(W, att_src, att_dst, bias):
    W = np.asarray(W, np.float32)
    att_src = np.asarray(att_src, np.float32)
    att_dst = np.asarray(att_dst, np.float32)
    bias = np.asarray(bias, np.float32)
    Wr = W.reshape(IN_DIM, HEADS, D_MODEL)
    a_src_cols = np.einsum("fhc,hc->fh", Wr, att_src)
    a_dst_cols = np.einsum("fhc,hc->fh", Wr, att_dst)
    # h channels stay (h, c)-major (natural W layout): col = h*D + c, so the
    # per-head epilogue reduce is a pair of half-width slice adds.
    waug = np.concatenate([W, a_src_cols, a_dst_cols], axis=1)
    biasrep = np.tile(bias[None, :], (P, 1)).astype(np.float32)
    t2row = np.tile(np.arange(P, dtype=ml_dtypes.bfloat16)[None, :], (P, 1)).copy()
    return {
        "waug": np.ascontiguousarray(waug, np.float32),
        "biasrep": biasrep,
        "t2row": t2row,
    }


def build_nc(nch, debug=False, num_devices=N_CORES):
    """Build the SPMD Bass program (identical across cores)."""
    nc = bacc.Bacc(
        "TRN2",
        target_bir_lowering=False,
        debug=debug,
        num_devices=num_devices,
        num_swdge_queues=4,
    )
    sum_nch = sum(nch)
    max_nch = max(nch)
    naug = HC + 2 * HEADS  # 264

    xt = nc.dram_tensor("xt", [P, S, N_PAD], BF16, kind="ExternalInput")
    waug = nc.dram_tensor("waug", [IN_DIM, naug], F32, kind="ExternalInput")
    biasrep = nc.dram_tensor("biasrep", [P, D_MODEL], F32, kind="ExternalInput")
    t2row = nc.dram_tensor("t2row", [P, P], BF16, kind="ExternalInput")
    dl = nc.dram_tensor("dl", [P, sum_nch], BF16, kind="ExternalInput")
    gidx = nc.dram_tensor("gidx", [P, sum_nch * 16], I16, kind="ExternalInput")
    hext = nc.dram_tensor("hext", [HEXT_ROWS, HEXT_W], BF16, kind="Internal")
    out = nc.dram_tensor("out", [OUT_ROWS, S, D_MODEL], F32, kind="ExternalOutput")

    with tile.TileContext(nc) as tc, ExitStack() as ctx:
        nc.gpsimd.load_library(library_config.mlp)
        tc.no_sync_barrier()

        consts = ctx.enter_context(tc.tile_pool(name="consts", bufs=1))
        waug_f32 = consts.tile([P, naug], F32)
        nc.sync.dma_start(waug_f32[:], waug[:, :])
        waug_t = consts.tile([P, naug], BF16)
        nc.vector.tensor_copy(waug_t[:], waug_f32[:])
        bias_t = consts.tile([P, D_MODEL], F32)
        nc.sync.dma_start(bias_t[:], biasrep[:, :])
        t2_t = consts.tile([P, P], BF16)
        nc.sync.dma_start(t2_t[:], t2row[:, :])
        dls = consts.tile([P, sum_nch], BF16)
        nc.sync.dma_start(dls[:], dl[:, :])

        # ---------------- phase 1: dense h + logits, all nodes x 4 steps ----
        h_scope = nc.enter_named_scope("h_phase", False)[0]
        with ExitStack() as p1:
            XG = 8  # node tiles per x load
            xpool = p1.enter_context(tc.tile_pool(name="x", bufs=2))
            stpool = p1.enter_context(tc.tile_pool(name="stage", bufs=3))
            ps1 = p1.enter_context(tc.tile_pool(name="ps1", bufs=2, space="PSUM"))

            for g0 in range(0, N_TILES, XG):
                gt = min(XG, N_TILES - g0)
                xg = xpool.tile([P, S, XG * P], BF16, tag="xg")
                nc.sync.dma_start(
                    xg[:, :, 0 : gt * P], xt[:, :, g0 * P : (g0 + gt) * P]
                )
                for t in range(gt):
                    m = g0 + t
                    ph = ps1.tile([P, S, 512], F32)
                    for s in range(S):
                        nc.tensor.matmul(
                            ph[:, s, 0:naug],
                            xg[:, s, t * P : (t + 1) * P],
                            waug_t[:],
                            start=True,
                            stop=True,
                        )
                    stage = stpool.tile([P, HEXT_W], BF16, tag="stage")
                    # h cast on the (otherwise idle in phase 1) scalar engine
                    nc.scalar.activation(
                        stage[:, 0 : S * HC].rearrange("p (s c) -> p s c", s=S),
                        ph[:, :, 0:HC],
                        mybir.ActivationFunctionType.Copy,
                    )
                    # aux: 32 f32, step-interleaved [a_src_s(4) | a_dst_s(4)]
                    nc.vector.tensor_copy(
                        stage[:, AUX_OFF : AUX_OFF + 64]
                        .bitcast(F32)
                        .rearrange("p (s v) -> p s v", s=S),
                        ph[:, :, HC:naug],
                    )
                    nc.sync.dma_start(hext[m * P : (m + 1) * P, :], stage[:])

            # dummy row for padded edge slots: a_src/a_dst = -1000 => ex == 0
            dstage = stpool.tile([P, HEXT_W], BF16, tag="stage")
            nc.vector.memset(dstage[:], 0.0)
            nc.vector.memset(
                dstage[:, AUX_OFF : AUX_OFF + 64].bitcast(F32), -1000.0
            )
            nc.sync.dma_start(hext[N_PAD : N_PAD + P, :], dstage[:])

        nc.leave_named_scope("h_phase", h_scope, False)
        tc.strict_bb_all_engine_barrier()

        # ---------------- phase 2: edge message passing ---------------------
        e_scope = nc.enter_named_scope("edge_phase", False)[0]

        hgp = ctx.enter_context(tc.tile_pool(name="hg", bufs=4))
        gixp = ctx.enter_context(tc.tile_pool(name="gix", bufs=3))
        arp = ctx.enter_context(tc.tile_pool(name="ar", bufs=2))
        exp_ = ctx.enter_context(tc.tile_pool(name="exf", bufs=4))
        msgp = ctx.enter_context(tc.tile_pool(name="msg", bufs=4))
        alp = ctx.enter_context(tc.tile_pool(name="al", bufs=4))
        ohp = ctx.enter_context(tc.tile_pool(name="oh", bufs=4))
        pep = ctx.enter_context(tc.tile_pool(name="pe", bufs=2, space="PSUM"))
        nump = ctx.enter_context(tc.tile_pool(name="num", bufs=2))
        wmp = ctx.enter_context(tc.tile_pool(name="wm", bufs=1))
        onp = ctx.enter_context(tc.tile_pool(name="on", bufs=2))

        lregs = {}
        for j in range(Q_TILES):
            for v in (min(SL, nch[j] - c0) * P for c0 in range(0, nch[j], SL)):
                lregs.setdefault(v, None)
            lregs.setdefault(nch[j] * P, None)
        for v in sorted(lregs):
            lregs[v] = nc.gpsimd.to_reg(v)

        chunk_base = 0
        num_t = None
        for j in range(Q_TILES):
            nj = nch[j]
            i0 = chunk_base * 16  # idx col offset (g1 at i0, g2 at i0+nj*8)

            gix = gixp.tile([P, max_nch * 16], I16)
            nc.sync.dma_start(gix[:, 0 : nj * 16], gidx[:, i0 : i0 + nj * 16])
            arow = arp.tile([P, max_nch, AUX_W], BF16)
            # <=1024 idxs per gather call (SWDGE descriptor ring capacity)
            for c0 in range(0, nj, SL):
                cc = min(SL, nj - c0)
                nc.gpsimd.dma_gather(
                    arow[:, c0 : c0 + cc, :],
                    hext[:, AUX_OFF : AUX_OFF + AUX_W],
                    gix[:, nj * 8 + c0 * 8 : nj * 8 + (c0 + cc) * 8],
                    cc * P,
                    lregs[cc * P],
                    AUX_W,
                    elem_step=HEXT_W,
                    queue_num=0,
                )

            pe = pep.tile([P, S, 512], F32)
            first = True
            for c0 in range(0, nj, SL):
                cc = min(SL, nj - c0)
                ll = cc * P
                hg = hgp.tile([P, SL, HEXT_W], BF16)
                nc.gpsimd.dma_gather(
                    hg[:, 0:cc, :],
                    hext[:, :],
                    gix[:, c0 * 8 : (c0 + cc) * 8],
                    ll,
                    lregs[ll],
                    HEXT_W,
                    queue_num=0,
                )

                # aux per row: 32 f32 step-interleaved [a_src_s | a_dst_s];
                # offset-by-4 aligns a_src[src] slots with a_dst[dst] slots
                # (slots 8s..8s+4 are alpha; slots 8s+4..8s+8 are garbage)
                alpha = alp.tile([P, SL, 2 * S * HEADS], F32, tag="alpha")
                nc.vector.tensor_add(
                    alpha[:, 0:cc, 0:28],
                    hg[:, 0:cc, AUX_OFF : AUX_OFF + 56].bitcast(F32),
                    arow[:, c0 : c0 + cc, 8:64].bitcast(F32),
                )
                lr = alp.tile([P, SL, 2 * S * HEADS], F32, tag="lr")
                nc.vector.scalar_tensor_tensor(
                    lr[:, 0:cc, 0:28],
                    alpha[:, 0:cc, 0:28],
                    NEG_SLOPE,
                    alpha[:, 0:cc, 0:28],
                    op0=mybir.AluOpType.mult,
                    op1=mybir.AluOpType.max,
                )
                exf = exp_.tile([P, SL, S, HC], BF16, tag="exf")
                nc.scalar.activation(
                    exf[:, 0:cc, :, :].rearrange(
                        "p n s (h c) -> p (n s) h c", h=HEADS
                    ),
                    lr[:, 0:cc, :]
                    .rearrange("p n (s v) -> p n s v", v=2 * HEADS)[:, :, :, 0:HEADS]
                    .rearrange("p n s h -> p (n s) h ()")
                    .broadcast_to((P, cc * S, HEADS, D_MODEL)),
                    mybir.ActivationFunctionType.Exp,
                )
                msg = msgp.tile([P, SL, S, MM_W], BF16)
                nc.vector.tensor_tensor(
                    msg[:, 0:cc, :, 0:HC].rearrange("p n s c -> p (n s) c"),
                    hg[:, 0:cc, 0 : S * HC],
                    exf[:, 0:cc, :, :].rearrange("p n s c -> p (n s) c"),
                    op=mybir.AluOpType.mult,
                )
                nc.vector.tensor_copy(
                    msg[:, 0:cc, :, HC:MM_W].rearrange("p n s h -> p (n s) h"),
                    exf[:, 0:cc, :, :].rearrange(
                        "p n s (h c) -> p (n s) h c", h=HEADS
                    )[:, :, :, 0:1],
                )
                oh = ohp.tile([P, SL, P], BF16)
                nc.vector.tensor_tensor(
                    oh[:, 0:cc, :],
                    t2_t[:].rearrange("p d -> p () d").broadcast_to((P, cc, P)),
                    dls[:, chunk_base + c0 : chunk_base + c0 + cc]
                    .rearrange("p n -> p n ()")
                    .broadcast_to((P, cc, P)),
                    op=mybir.AluOpType.is_equal,
                )
                for ch in range(cc):
                    last = c0 + ch == nj - 1
                    for s in range(S):
                        nc.tensor.matmul(
                            pe[:, s, 0:MM_W],
                            oh[:, ch, :],
                            msg[:, ch, s, :],
                            start=first,
                            stop=last,
                        )
                    first = False

            g = j % EPI_G
            if g == 0:
                num_t = nump.tile([P, EPI_G, S, MM_W], F32)
            nc.vector.tensor_copy(num_t[:, g, :, :], pe[:, :, 0:MM_W])

            if g == EPI_G - 1:
                j0 = j - (EPI_G - 1)
                r = alp.tile([P, EPI_G, S, HEADS], F32, tag="r")
                nc.vector.reciprocal(r[:], num_t[:, :, :, HC:MM_W])
                # broadcast-expand (1/den)/4 per head on the scalar engine
                rexp = wmp.tile([P, EPI_G, S, HC], F32, tag="rexp")
                nc.scalar.activation(
                    rexp[:].rearrange("p g s (h c) -> p (g s) h c", h=HEADS),
                    r[:]
                    .rearrange("p g s h -> p (g s) h ()")
                    .broadcast_to((P, EPI_G * S, HEADS, D_MODEL)),
                    mybir.ActivationFunctionType.Copy,
                    scale=1.0 / HEADS,
                )
                wm = num_t[:, :, :, 0:HC]  # in-place: num *= rexp
                nc.vector.tensor_tensor(
                    wm, wm, rexp[:], op=mybir.AluOpType.mult
                )
                # head mean: (h,c)-major -> two half-width slice adds
                th = wmp.tile([P, EPI_G, S, 2 * D_MODEL], F32, tag="th")
                nc.vector.tensor_add(
                    th[:],
                    wm[:, :, :, 0 : 2 * D_MODEL],
                    wm[:, :, :, 2 * D_MODEL : 4 * D_MODEL],
                )
                onode = onp.tile([P, EPI_G, S, D_MODEL], F32, tag="onode")
                nc.vector.tensor_add(
                    onode[:], th[:, :, :, 0:D_MODEL], th[:, :, :, D_MODEL:]
                )
                nc.vector.tensor_tensor(
                    onode[:].rearrange("p g s c -> p (g s) c"),
                    onode[:].rearrange("p g s c -> p (g s) c"),
                    bias_t[:]
                    .rearrange("p c -> p () c")
                    .broadcast_to((P, EPI_G * S, D_MODEL)),
                    op=mybir.AluOpType.add,
                )
                nc.sync.dma_start(
                    out[j0 * P : (j + 1) * P, :, :].rearrange(
                        "(g p) s c -> p g (s c)", p=P
                    ),
                    onode[:].rearrange("p g s c -> p g (s c)"),
                )
            chunk_base += nj

        nc.leave_named_scope("edge_phase", e_scope, False)

    # Spread gathers over the 4 SWDGE queues. Each DMASW sem lane is locked to
    # one queue, so derive the queue from the lane Tile assigned (k % 4).
    import re

    for f in nc.m.functions:
        for bb in f.blocks:
            for inst in bb.instructions:
                if isinstance(inst, mybir.InstDMAGatherAnt):
                    si = inst.sync_info
                    if si and si.on_update:
                        name = getattr(si.on_update[0], "ant_name", "") or ""
                        mt = re.match(r"DMASW(\d+)", name)
                        if mt:
                            inst.queue_num = int(mt.group(1)) % 4

    nc.compile()
    return nc


_CACHE = {}


def _prepare(x, edge_index, W, att_src, att_dst, bias):
    x = np.asarray(x, np.float32)
    key = hash(np.asarray(edge_index).tobytes())
    if key not in _CACHE:
        nch, per_quarter = preprocess_edges(edge_index)
        nc = build_nc(nch, debug=False, num_devices=N_CORES)
        _CACHE.clear()
        _CACHE[key] = (nc, nch, per_quarter)
    nc, nch, per_quarter = _CACHE[key]
    consts = build_consts(W, att_src, att_dst, bias)
    # x [N, T, F] f32 -> per step-quad [F, S, N_PAD] bf16
    xq = []
    for t0 in (0, S):
        xp = np.zeros((IN_DIM, S, N_PAD), ml_dtypes.bfloat16)
        xp[:, :, 0:N_NODES] = x[:, t0 : t0 + S, :].transpose(2, 1, 0).astype(
            ml_dtypes.bfloat16
        )
        xq.append(np.ascontiguousarray(xp))
    in_maps = []
    for c in range(N_CORES):
        q = c % 4
        gidx, dl_all = per_quarter[q]
        in_maps.append(
            {
                "xt": xq[c // 4],
                "gidx": gidx,
                "dl": dl_all,
                **consts,
            }
        )
    return nc, in_maps


def _assemble(res):
    out = np.empty((N_NODES, T_STEPS, D_MODEL), np.float32)
    for c in range(N_CORES):
        q = c % 4
        t0 = (c // 4) * S
        n0, n1 = QN_BOUNDS[q], QN_BOUNDS[q + 1]
        core_out = res.results[c]["out"]  # [OUT_ROWS, S, 64]
        out[n0:n1, t0 : t0 + S, :] = core_out[0 : n1 - n0]
    return out


def kernel(x, edge_index, W, att_src, att_dst, bias):
    nc, in_maps = _prepare(x, edge_index, W, att_src, att_dst, bias)
    res = run_bass_kernel_spmd(nc, in_maps, core_ids=list(range(N_CORES)))
    return _assemble(res)


def kernel_profiled(x, edge_index, W, att_src, att_dst, bias):
    """Run with NTFF tracing; returns (output, exec_time_ns, results obj)."""
    nc, in_maps = _prepare(x, edge_index, W, att_src, att_dst, bias)
    res = run_bass_kernel_spmd(
        nc, in_maps, core_ids=list(range(N_CORES)), trace=True
    )
    return _assemble(res), res.exec_time_ns, res


# revision 31
# speedup vs baseline: 1.6740x; 1.0597x over previous
"""Self-contained Trainium2 Bass kernel for a batched (time-stepped) GAT layer.

Problem: x [N=20000, T=8, F=128], edge_index [2, E=320000] (+self loops),
W [128, 256] (4 heads x 64), att_src/att_dst [4, 64], bias [64].
Per time step: GATConv (concat=False -> head mean) with softmax attention.
Output: [N, T, 64] f32.

Sharding (8 cores): 2 step-quads x 4 node-quarters. Each core handles 4 time
steps for ~5000 destination nodes. The per-edge h[src] gather row packs all 4
steps (2304B), so gather descriptor count (the gpsimd/SWDGE bottleneck) drops
4x vs one-step rows, and every per-edge vector op is batched across steps.

Per-core algorithm:
  Phase 1 (dense, all 157 node tiles x 4 steps): h_s = x_s @ W_aug where W_aug
    also yields per-node a_src/a_dst logits. Rows written to HBM 'hext'
    [n, 1152 bf16]: 4x256 h (c,h)-major | 4x4 a_src f32 | 4x4 a_dst f32 | pad.
  Phase 2 (edges of our quarter, sorted by destination, per 128-dst tile,
    sliced into 8-chunk pieces):
    - dma_gather hext rows by src (2304B)     -> h[src], a_src[src]
    - dma_gather hext tail 256B slices by dst -> a_dst[dst]
    - alpha = leaky_relu(a_src+a_dst) for 4 steps x 4 heads in 2 DVE ops
    - ex = exp(alpha) broadcast-expanded on the scalar engine to (s, c, h)
    - msg = h * ex (one DVE op over all 4 steps, 2x bf16 mode)
    - one-hot(dst_local) matmuls accumulate per-step segment sums in PSUM:
      numerator [128, 256] + denominator [128, 4] per step
    - batched epilogue every 4 tiles: out = (num/den).mean(heads) + bias
"""

import numpy as np
import ml_dtypes
from contextlib import ExitStack

import concourse.bass as bass
import concourse.bacc as bacc
import concourse.mybir as mybir
import concourse.tile as tile
from concourse import library_config
from concourse.bass_utils import run_bass_kernel_spmd

F32 = mybir.dt.float32
BF16 = mybir.dt.bfloat16
I16 = mybir.dt.int16

P = 128
N_NODES = 20000
IN_DIM = 128
HEADS = 4
D_MODEL = 64
HC = HEADS * D_MODEL          # 256
T_STEPS = 8
S = 4                         # time steps packed per core
NEG_SLOPE = 0.2
N_CORES = 8

N_TILES = (N_NODES + P - 1) // P          # 157
N_PAD = N_TILES * P                        # 20096
DUMMY_ROW = N_PAD                          # a_dst = -1000 -> ex == 0
HEXT_ROWS = N_PAD + P
# bf16 cols: 4*256 h | 32 (4x4 a_src f32) | 32 (4x4 a_dst f32) | pad
HEXT_W = S * HC + P                        # 1152 cols = 2304 B
AUX_OFF = S * HC                           # 1024 (bf16 col of a_src block)
AUX_W = P                                  # 256B tail slice for the dst gather
MM_W = HC + HEADS                          # 260 matmul rhs width per step

Q_TILES = 40                               # tiles per quarter (ghost-padded)
QT_BOUNDS = [0, 40, 79, 118, 157]          # quarter tile boundaries
QN_BOUNDS = [0, 5120, 10112, 15104, 20000]  # quarter node boundaries
OUT_ROWS = Q_TILES * P                     # 5120 rows per core (tail = scratch)
SL = 4                                     # chunks per gather slice
EPI_G = 4                                  # tiles per batched epilogue group


def preprocess_edges(edge_index):
    """Sort (edges + self loops) by destination; build per-quarter gather
    indices with tile shapes equalized across quarters (SPMD: all cores run
    the identical program; only the index *contents* differ per core).

    Returns (nch: [Q_TILES] chunks per local tile, per_quarter: list of
    (gidx [128, sum_nch*16] int16, dl [128, sum_nch] bf16)).
    """
    loops = np.arange(N_NODES, dtype=np.int64)
    src = np.concatenate([np.asarray(edge_index[0], dtype=np.int64), loops])
    dst = np.concatenate([np.asarray(edge_index[1], dtype=np.int64), loops])
    order = np.argsort(dst, kind="stable")
    src_s = src[order]
    dst_s = dst[order]
    counts = np.bincount(dst_s // P, minlength=N_TILES)
    starts = np.concatenate([[0], np.cumsum(counts)])

    # equalized chunks per local tile index
    nch = np.ones(Q_TILES, np.int64)
    for q in range(4):
        for j in range(QT_BOUNDS[q + 1] - QT_BOUNDS[q]):
            g = QT_BOUNDS[q] + j
            nch[j] = max(nch[j], (counts[g] + P - 1) // P)

    def wrap(flat):
        w = flat.reshape(-1, 16).T.copy()
        return np.tile(w, (8, 1)).copy()

    per_quarter = []
    for q in range(4):
        g1_parts, g2_parts, dl_parts = [], [], []
        for j in range(Q_TILES):
            g = QT_BOUNDS[q] + j
            lpad = int(nch[j]) * P
            g1 = np.zeros(lpad, np.int16)
            g2 = np.full(lpad, DUMMY_ROW, np.int16)
            dl = np.full(lpad, 200.0, np.float64)
            if g < QT_BOUNDS[q + 1]:
                length = int(counts[g])
                pos = int(starts[g])
                g1[:length] = src_s[pos : pos + length]
                g2[:length] = dst_s[pos : pos + length]
                dl[:length] = (dst_s[pos : pos + length] - g * P).astype(np.float64)
            g1_parts.append(wrap(g1))
            g2_parts.append(wrap(g2))
            dl_parts.append(dl.reshape(-1, P).T.astype(ml_dtypes.bfloat16))
        # per tile: [g1 | g2] so one resident idx tensor serves both gathers
        gidx = np.concatenate(
            [np.concatenate([a, b], axis=1) for a, b in zip(g1_parts, g2_parts)],
            axis=1,
        )
        dl_all = np.concatenate(dl_parts, axis=1)
        # host-built one-hot(dst_local): [128 edge-lanes, sum_nch*128 dst-cols]
        oh = (
            dl_all.astype(np.int32)[:, :, None] == np.arange(P, dtype=np.int32)
        ).astype(ml_dtypes.bfloat16)
        per_quarter.append(
            (
                np.ascontiguousarray(gidx),
                np.ascontiguousarray(oh.reshape(P, -1)),
            )
        )
    return nch.tolist(), per_quarter


def build_consts(W, att_src, att_dst, bias):
    W = np.asarray(W, np.float32)
    att_src = np.asarray(att_src, np.float32)
    att_dst = np.asarray(att_dst, np.float32)
    bias = np.asarray(bias, np.float32)
    Wr = W.reshape(IN_DIM, HEADS, D_MODEL)
    a_src_cols = np.einsum("fhc,hc->fh", Wr, att_src)
    a_dst_cols = np.einsum("fhc,hc->fh", Wr, att_dst)
    # h channels stay (h, c)-major (natural W layout): col = h*D + c, so the
    # per-head epilogue reduce is a pair of half-width slice adds.
    waug = np.concatenate([W, a_src_cols, a_dst_cols], axis=1)
    biasrep = np.tile(bias[None, :], (P, 1)).astype(np.float32)
    return {
        "waug": np.ascontiguousarray(waug, np.float32),
        "biasrep": biasrep,
    }


def build_nc(nch, debug=False, num_devices=N_CORES):
    """Build the SPMD Bass program (identical across cores)."""
    nc = bacc.Bacc(
        "TRN2",
        target_bir_lowering=False,
        debug=debug,
        num_devices=num_devices,
        num_swdge_queues=4,
    )
    sum_nch = sum(nch)
    max_nch = max(nch)
    naug = HC + 2 * HEADS  # 264

    xt = nc.dram_tensor("xt", [P, S, N_PAD], BF16, kind="ExternalInput")
    waug = nc.dram_tensor("waug", [IN_DIM, naug], F32, kind="ExternalInput")
    biasrep = nc.dram_tensor("biasrep", [P, D_MODEL], F32, kind="ExternalInput")
    ohs = nc.dram_tensor("ohs", [P, sum_nch * P], BF16, kind="ExternalInput")
    gidx = nc.dram_tensor("gidx", [P, sum_nch * 16], I16, kind="ExternalInput")
    hext = nc.dram_tensor("hext", [HEXT_ROWS, HEXT_W], BF16, kind="Internal")
    out = nc.dram_tensor("out", [OUT_ROWS, S, D_MODEL], F32, kind="ExternalOutput")

    with tile.TileContext(nc) as tc, ExitStack() as ctx:
        nc.gpsimd.load_library(library_config.mlp)
        tc.no_sync_barrier()

        consts = ctx.enter_context(tc.tile_pool(name="consts", bufs=1))
        waug_f32 = consts.tile([P, naug], F32)
        nc.sync.dma_start(waug_f32[:], waug[:, :])
        waug_t = consts.tile([P, naug], BF16)
        nc.vector.tensor_copy(waug_t[:], waug_f32[:])
        bias_t = consts.tile([P, D_MODEL], F32)
        nc.sync.dma_start(bias_t[:], biasrep[:, :])

        # ---------------- phase 1: dense h + logits, all nodes x 4 steps ----
        h_scope = nc.enter_named_scope("h_phase", False)[0]
        with ExitStack() as p1:
            XG = 8  # node tiles per x load
            xpool = p1.enter_context(tc.tile_pool(name="x", bufs=3))
            stpool = p1.enter_context(tc.tile_pool(name="stage", bufs=4))
            ps1 = p1.enter_context(tc.tile_pool(name="ps1", bufs=2, space="PSUM"))

            for g0 in range(0, N_TILES, XG):
                gt = min(XG, N_TILES - g0)
                xg = xpool.tile([P, S, XG * P], BF16, tag="xg")
                nc.sync.dma_start(
                    xg[:, :, 0 : gt * P], xt[:, :, g0 * P : (g0 + gt) * P]
                )
                for t in range(gt):
                    m = g0 + t
                    ph = ps1.tile([P, S, 512], F32)
                    for s in range(S):
                        nc.tensor.matmul(
                            ph[:, s, 0:naug],
                            xg[:, s, t * P : (t + 1) * P],
                            waug_t[:],
                            start=True,
                            stop=True,
                        )
                    stage = stpool.tile([P, HEXT_W], BF16, tag="stage")
                    # h cast: alternate DVE/ACT so neither paces phase 1
                    if m % 2 == 0:
                        nc.scalar.activation(
                            stage[:, 0 : S * HC].rearrange("p (s c) -> p s c", s=S),
                            ph[:, :, 0:HC],
                            mybir.ActivationFunctionType.Copy,
                        )
                    else:
                        nc.vector.tensor_copy(
                            stage[:, 0 : S * HC].rearrange("p (s c) -> p s c", s=S),
                            ph[:, :, 0:HC],
                        )
                    # aux: 32 f32, step-interleaved [a_src_s(4) | a_dst_s(4)]
                    nc.vector.tensor_copy(
                        stage[:, AUX_OFF : AUX_OFF + 64]
                        .bitcast(F32)
                        .rearrange("p (s v) -> p s v", s=S),
                        ph[:, :, HC:naug],
                    )
                    nc.sync.dma_start(hext[m * P : (m + 1) * P, :], stage[:])

            # dummy row for padded edge slots: a_src/a_dst = -1000 => ex == 0
            dstage = stpool.tile([P, HEXT_W], BF16, tag="stage")
            nc.vector.memset(dstage[:], 0.0)
            nc.vector.memset(
                dstage[:, AUX_OFF : AUX_OFF + 64].bitcast(F32), -1000.0
            )
            nc.sync.dma_start(hext[N_PAD : N_PAD + P, :], dstage[:])

        nc.leave_named_scope("h_phase", h_scope, False)
        tc.strict_bb_all_engine_barrier()

        # ---------------- phase 2: edge message passing ---------------------
        e_scope = nc.enter_named_scope("edge_phase", False)[0]

        hgp = ctx.enter_context(tc.tile_pool(name="hg", bufs=5))
        gixp = ctx.enter_context(tc.tile_pool(name="gix", bufs=3))
        arp = ctx.enter_context(tc.tile_pool(name="ar", bufs=2))
        exp_ = ctx.enter_context(tc.tile_pool(name="exf", bufs=5))
        alp = ctx.enter_context(tc.tile_pool(name="al", bufs=4))
        ohp = ctx.enter_context(tc.tile_pool(name="oh", bufs=4))
        pep = ctx.enter_context(tc.tile_pool(name="pe", bufs=2, space="PSUM"))
        nump = ctx.enter_context(tc.tile_pool(name="num", bufs=2))
        wmp = ctx.enter_context(tc.tile_pool(name="wm", bufs=1))
        onp = ctx.enter_context(tc.tile_pool(name="on", bufs=2))

        lregs = {}
        for j in range(Q_TILES):
            for v in (min(SL, nch[j] - c0) * P for c0 in range(0, nch[j], SL)):
                lregs.setdefault(v, None)
            lregs.setdefault(nch[j] * P, None)
        for v in sorted(lregs):
            lregs[v] = nc.gpsimd.to_reg(v)

        chunk_base = 0
        num_t = None
        for j in range(Q_TILES):
            nj = nch[j]
            i0 = chunk_base * 16  # idx col offset (g1 at i0, g2 at i0+nj*8)

            gix = gixp.tile([P, max_nch * 16], I16)
            nc.sync.dma_start(gix[:, 0 : nj * 16], gidx[:, i0 : i0 + nj * 16])
            arow = arp.tile([P, max_nch, AUX_W], BF16)
            # <=1024 idxs per gather call (SWDGE descriptor ring capacity)
            for c0 in range(0, nj, SL):
                cc = min(SL, nj - c0)
                nc.gpsimd.dma_gather(
                    arow[:, c0 : c0 + cc, :],
                    hext[:, AUX_OFF : AUX_OFF + AUX_W],
                    gix[:, nj * 8 + c0 * 8 : nj * 8 + (c0 + cc) * 8],
                    cc * P,
                    lregs[cc * P],
                    AUX_W,
                    elem_step=HEXT_W,
                    queue_num=0,
                )

            pe = pep.tile([P, S, 512], F32)
            first = True
            for c0 in range(0, nj, SL):
                cc = min(SL, nj - c0)
                ll = cc * P
                hg = hgp.tile([P, SL, HEXT_W], BF16)
                nc.gpsimd.dma_gather(
                    hg[:, 0:cc, :],
                    hext[:, :],
                    gix[:, c0 * 8 : (c0 + cc) * 8],
                    ll,
                    lregs[ll],
                    HEXT_W,
                    queue_num=0,
                )

                # aux per row: 32 f32 step-interleaved [a_src_s | a_dst_s];
                # offset-by-4 aligns a_src[src] slots with a_dst[dst] slots
                # (slots 8s..8s+4 are alpha; slots 8s+4..8s+8 are garbage)
                alpha = alp.tile([P, SL, 2 * S * HEADS], F32, tag="alpha")
                nc.vector.tensor_add(
                    alpha[:, 0:cc, 0:28],
                    hg[:, 0:cc, AUX_OFF : AUX_OFF + 56].bitcast(F32),
                    arow[:, c0 : c0 + cc, 8:64].bitcast(F32),
                )
                lr = alp.tile([P, SL, 2 * S * HEADS], F32, tag="lr")
                nc.vector.scalar_tensor_tensor(
                    lr[:, 0:cc, 0:28],
                    alpha[:, 0:cc, 0:28],
                    NEG_SLOPE,
                    alpha[:, 0:cc, 0:28],
                    op0=mybir.AluOpType.mult,
                    op1=mybir.AluOpType.max,
                )
                # exf becomes msg in place: ACT writes exp(alpha) expanded to
                # all channels plus the raw ex denominator columns (256:260),
                # then DVE multiplies the h part by hg in place.
                exf = exp_.tile([P, SL, S, MM_W], BF16, tag="exf")
                nc.scalar.activation(
                    exf[:, 0:cc, :, 0:HC].rearrange(
                        "p n s (h c) -> p (n s) h c", h=HEADS
                    ),
                    lr[:, 0:cc, :]
                    .rearrange("p n (s v) -> p n s v", v=2 * HEADS)[:, :, :, 0:HEADS]
                    .rearrange("p n s h -> p (n s) h ()")
                    .broadcast_to((P, cc * S, HEADS, D_MODEL)),
                    mybir.ActivationFunctionType.Exp,
                )
                nc.scalar.activation(
                    exf[:, 0:cc, :, HC:MM_W].rearrange("p n s h -> p (n s) h"),
                    lr[:, 0:cc, :]
                    .rearrange("p n (s v) -> p n s v", v=2 * HEADS)[:, :, :, 0:HEADS]
                    .rearrange("p n s h -> p (n s) h"),
                    mybir.ActivationFunctionType.Exp,
                )
                nc.vector.tensor_tensor(
                    exf[:, 0:cc, :, 0:HC].rearrange("p n s c -> p (n s) c"),
                    exf[:, 0:cc, :, 0:HC].rearrange("p n s c -> p (n s) c"),
                    hg[:, 0:cc, 0 : S * HC],
                    op=mybir.AluOpType.mult,
                )
                oh = ohp.tile([P, SL, P], BF16)
                nc.sync.dma_start(
                    oh[:, 0:cc, :],
                    ohs[:, (chunk_base + c0) * P : (chunk_base + c0 + cc) * P],
                )
                for ch in range(cc):
                    last = c0 + ch == nj - 1
                    for s in range(S):
                        nc.tensor.matmul(
                            pe[:, s, 0:MM_W],
                            oh[:, ch, :],
                            exf[:, ch, s, :],
                            start=first,
                            stop=last,
                        )
                    first = False

            g = j % EPI_G
            if g == 0:
                num_t = nump.tile([P, EPI_G, S, MM_W], F32)
            nc.vector.tensor_copy(num_t[:, g, :, :], pe[:, :, 0:MM_W])

            if g == EPI_G - 1:
                j0 = j - (EPI_G - 1)
                r = alp.tile([P, EPI_G, S, HEADS], F32, tag="r")
                nc.vector.reciprocal(r[:], num_t[:, :, :, HC:MM_W])
                # broadcast-expand (1/den)/4 per head on the scalar engine
                rexp = wmp.tile([P, EPI_G, S, HC], F32, tag="rexp")
                nc.scalar.activation(
                    rexp[:].rearrange("p g s (h c) -> p (g s) h c", h=HEADS),
                    r[:]
                    .rearrange("p g s h -> p (g s) h ()")
                    .broadcast_to((P, EPI_G * S, HEADS, D_MODEL)),
                    mybir.ActivationFunctionType.Copy,
                    scale=1.0 / HEADS,
                )
                wm = num_t[:, :, :, 0:HC]  # in-place: num *= rexp
                nc.vector.tensor_tensor(
                    wm, wm, rexp[:], op=mybir.AluOpType.mult
                )
                # head mean: (h,c)-major -> two half-width slice adds
                th = wmp.tile([P, EPI_G, S, 2 * D_MODEL], F32, tag="th")
                nc.vector.tensor_add(
                    th[:],
                    wm[:, :, :, 0 : 2 * D_MODEL],
                    wm[:, :, :, 2 * D_MODEL : 4 * D_MODEL],
                )
                onode = onp.tile([P, EPI_G, S, D_MODEL], F32, tag="onode")
                nc.vector.tensor_add(
                    onode[:], th[:, :, :, 0:D_MODEL], th[:, :, :, D_MODEL:]
                )
                nc.vector.tensor_tensor(
                    onode[:].rearrange("p g s c -> p (g s) c"),
                    onode[:].rearrange("p g s c -> p (g s) c"),
                    bias_t[:]
                    .rearrange("p c -> p () c")
                    .broadcast_to((P, EPI_G * S, D_MODEL)),
                    op=mybir.AluOpType.add,
                )
                nc.sync.dma_start(
                    out[j0 * P : (j + 1) * P, :, :].rearrange(
                        "(g p) s c -> p g (s c)", p=P
                    ),
                    onode[:].rearrange("p g s c -> p g (s c)"),
                )
            chunk_base += nj

        nc.leave_named_scope("edge_phase", e_scope, False)

    # Spread gathers over the 4 SWDGE queues. Each DMASW sem lane is locked to
    # one queue, so derive the queue from the lane Tile assigned (k % 4).
    import re

    for f in nc.m.functions:
        for bb in f.blocks:
            for inst in bb.instructions:
                if isinstance(inst, mybir.InstDMAGatherAnt):
                    si = inst.sync_info
                    if si and si.on_update:
                        name = getattr(si.on_update[0], "ant_name", "") or ""
                        mt = re.match(r"DMASW(\d+)", name)
                        if mt:
                            inst.queue_num = int(mt.group(1)) % 4

    nc.compile()
    return nc


_CACHE = {}


def _prepare(x, edge_index, W, att_src, att_dst, bias):
    x = np.asarray(x, np.float32)
    key = hash(np.asarray(edge_index).tobytes())
    if key not in _CACHE:
        nch, per_quarter = preprocess_edges(edge_index)
        nc = build_nc(nch, debug=False, num_devices=N_CORES)
        _CACHE.clear()
        _CACHE[key] = (nc, nch, per_quarter)
    nc, nch, per_quarter = _CACHE[key]
    consts = build_consts(W, att_src, att_dst, bias)
    # x [N, T, F] f32 -> per step-quad [F, S, N_PAD] bf16
    xq = []
    for t0 in (0, S):
        xp = np.zeros((IN_DIM, S, N_PAD), ml_dtypes.bfloat16)
        xp[:, :, 0:N_NODES] = x[:, t0 : t0 + S, :].transpose(2, 1, 0).astype(
            ml_dtypes.bfloat16
        )
        xq.append(np.ascontiguousarray(xp))
    in_maps = []
    for c in range(N_CORES):
        q = c % 4
        gidx, oh = per_quarter[q]
        in_maps.append(
            {
                "xt": xq[c // 4],
                "gidx": gidx,
                "ohs": oh,
                **consts,
            }
        )
    return nc, in_maps


def _assemble(res):
    out = np.empty((N_NODES, T_STEPS, D_MODEL), np.float32)
    for c in range(N_CORES):
        q = c % 4
        t0 = (c // 4) * S
        n0, n1 = QN_BOUNDS[q], QN_BOUNDS[q + 1]
        core_out = res.results[c]["out"]  # [OUT_ROWS, S, 64]
        out[n0:n1, t0 : t0 + S, :] = core_out[0 : n1 - n0]
    return out


def kernel(x, edge_index, W, att_src, att_dst, bias):
    nc, in_maps = _prepare(x, edge_index, W, att_src, att_dst, bias)
    res = run_bass_kernel_spmd(nc, in_maps, core_ids=list(range(N_CORES)))
    return _assemble(res)


def kernel_profiled(x, edge_index, W, att_src, att_dst, bias):
    """Run with NTFF tracing; returns (output, exec_time_ns, results obj)."""
    nc, in_maps = _prepare(x, edge_index, W, att_src, att_dst, bias)
    res = run_bass_kernel_spmd(
        nc, in_maps, core_ids=list(range(N_CORES)), trace=True
    )
    return _assemble(res), res.exec_time_ns, res
